# revision 20
# baseline (speedup 1.0000x reference)
"""Trainium2 Bass kernel for the LUT-linear (embedding_lookup) problem.

Math: per_table[b,t] = sum_c lut[t,c] * prod_j (1 + s_{c,j} x_j)/2 with
x_0 = input[b, mask[2t]], x_1 = input[b, mask[2t+1]], K=2 (KK=4 corners).
Expanding the corner products (codes s in {-1,+1}):
    per_table = a_t + b_t x0 + c_t x1 + d_t x0 x1
    4a = w0+w1+w2+w3, 4b = -w0+w1-w2+w3, 4c = -w0-w1+w2+w3, 4d = w0-w1-w2+w3
out[b,o] = bias[o] + sum_{t in seg_o} per_table   (segments are 512 contiguous
tables per out-feature).

Device strategy (8 NeuronCores, table-sharded; input replicated):
  - per core: 32768 tables = 64 out-features. Gather x0/x1 columns with
    SWDGE dma_gather from a bf16 padded input^T [512, 128] (256B rows, the
    top 64 lanes zero); each descriptor moves all 64 batch values of one
    input feature into per-chunk SBUF tiles [128 part, wc, 128]
    (partition = table mod 128, free = batch). Gathers are 1024 indices
    each (ucode ring limit) spread round-robin over 4 SWDGE queues.
  - Work is split into 7 full chunks (32 slots) + 4 small tail chunks
    (8 slots) so the post-last-gather compute tail stays short. One
    gather tile PER CHUNK keeps dependency ranges chunk-local.
  - ACT (ScalarE) compacts each full chunk's padded [.,.,128] gather
    tiles to contiguous [128, wc, 64] (x1 first - the first DVE op only
    needs x1). DVE computes y = x0*(b + d*x1) + c*x1 in bf16; with
    contiguous operands the tensor_tensor ADDs run 2x (bf16 MULTs are
    1x - no 2x uop exists for TT mult on cayman). Small tail chunks
    skip compaction (ACT off the tail critical path).
  - All segment reductions run on the PE: per chunk, matmul
    psum[64o, 8w-window, 64b] += pm^T @ y accumulates across chunks
    (window = global slot mod 32); one final strided tensor_reduce over
    w collapses psum to [64, 64]. The constant term a reduces on DVE
    once (one op over all chunks) and joins via a tiny [128,1] matmul.
  - Host does only data-independent layout transforms (transpose, cast,
    permute, shard) and the final unshard.
  - Measured: ~186.1 us HW exec (baseline of this architecture was
    186.99 us), rel err 4.2e-3. Breakdown: head ~16.7 us (runtime
    preamble ~3.5 + mlp library LOAD_LIB + lazy Q7 IRAM load ~6 + first
    gather latency), gather stream ~149 us, tail ~17 us, postamble.
  - The wall is the SWDGE gather stream: 128 dma_gather calls at
    ~2.33 ns/index = ~149 us. This is the Q7 CounterMachine descriptor
    push rate (TX and RX cores each push num_idxs 64B descriptors in
    parallel; cross-queue overlap is impossible because each extended
    instruction occupies all 8 Q7 cores and queue_num only selects the
    working pair). single_packet=True already coalesces the per-engine
    descriptor stream into one packet, so the SDMA small-descriptor HBM
    penalty is amortized; SDMA drain is NOT the wall. Dead ends
    explored: one-hot+PE gather (PE matmul instruction overhead
    ~427 ns/<=1024-col matmul makes it >= the SWDGE rate per table; DVE
    is_equal one-hot build alone costs 2.2 ns/table), ap_gather /
    indirect_copy (per-element Q7 compute gathers, ~100x too slow),
    custom 4-pair ucode (no Xtensa toolchain in container).
    NOTES: (1) x0/x1 pools live on SBUF side="right" - when they sit
    near the pinned DynamicDMAScratch carveout (addr 0), SDMA
    descriptor-ring reads conflict with gather-data writes (~218-226us).
    (2) tensor_reduce is always 1x; PSUM matmul output <= 1 bank
    (512 f32). (3) DVE clock 0.96 GHz; TT cost ~ (N/accel + 151)/0.96 ns.
"""

import numpy as np

NCORES = 8
B = 64
IN = 512
OUT = 512
T = IN * OUT
TC = T // NCORES          # tables per core = 32768
SEG = 512                 # tables per out-feature
OC = OUT // NCORES        # out-features per core = 64
NPART = 128
WT = TC // NPART          # tables per partition total = 256

# tuning knobs
NCHUNK = 8                # compute chunks per core
W = WT // NCHUNK          # tables per partition per chunk
TCHUNK = NPART * W        # tables per chunk
GIDX = 1024               # indices per dma_gather (ucode limit)
GSUB = TCHUNK // GIDX     # sub-gathers per compute chunk
GW = GIDX // NPART        # tables per partition per sub-gather
NQUEUES = 4
EPAD = 128                # padded batch row (bf16 elems) = 256B descriptor

_CACHE = {}


def _build_program():
    import concourse.bacc as bacc
    import concourse.mybir as mybir
    from concourse import library_config
    from concourse.tile import TileContext

    f32 = mybir.dt.float32
    bf16 = mybir.dt.bfloat16
    i16 = mybir.dt.int16
    Alu = mybir.AluOpType
    Axis = mybir.AxisListType

    S = TCHUNK // 16      # idx columns per chunk (16-partition wrap)

    nc = bacc.Bacc("TRN2", target_bir_lowering=False, debug=False,
                   num_devices=NCORES, num_swdge_queues=NQUEUES,
                   dynamic_dma_scratch_size=32768)

    input_t = nc.dram_tensor("input_t", [IN, EPAD], bf16, kind="ExternalInput")
    idx0_d = nc.dram_tensor("idx0", [NPART, NCHUNK * S], i16, kind="ExternalInput")
    idx1_d = nc.dram_tensor("idx1", [NPART, NCHUNK * S], i16, kind="ExternalInput")
    lutp_d = nc.dram_tensor("lutp", [NPART, NCHUNK, W * 4], bf16, kind="ExternalInput")
    bias_d = nc.dram_tensor("bias_sh", [OC, 1], f32, kind="ExternalInput")
    pm_d = nc.dram_tensor("pm", [NPART, OC], bf16, kind="ExternalInput")
    out_d = nc.dram_tensor("out_c", [OC, B], f32, kind="ExternalOutput")

    with TileContext(nc) as tc:
        nc.gpsimd.load_library(library_config.mlp)
        with (
            tc.tile_pool(name="idx", bufs=1) as idx_pool,
            tc.tile_pool(name="small", bufs=1) as small_pool,
            tc.tile_pool(name="lut", bufs=1) as lut_pool,
            tc.tile_pool(name="coef", bufs=1) as coef_pool,
            tc.tile_pool(name="x0", bufs=1, side="right") as x0_pool,
            tc.tile_pool(name="x1", bufs=1, side="right") as x1_pool,
            tc.tile_pool(name="m", bufs=2) as m_pool,
            tc.tile_pool(name="red", bufs=2) as red_pool,
            tc.tile_pool(name="psum", bufs=1, space="PSUM") as psum_pool,
        ):
            idx0_sb = idx_pool.tile([NPART, NCHUNK * S], i16, tag="idx0")
            idx1_sb = idx_pool.tile([NPART, NCHUNK * S], i16, tag="idx1")
            # per-chunk slice loads so the first gather only waits ~64KB
            for c in range(NCHUNK):
                sl = slice(c * S, (c + 1) * S)
                nc.sync.dma_start(idx0_sb[:, sl], idx0_d[:, sl])
                nc.sync.dma_start(idx1_sb[:, sl], idx1_d[:, sl])

            pm_sb = small_pool.tile([NPART, OC], bf16, tag="pm")
            nc.sync.dma_start(pm_sb[:], pm_d[:])
            bias_sb = small_pool.tile([OC, 1], f32, tag="bias")
            nc.sync.dma_start(bias_sb[:], bias_d[:])

            # full coefficient table, one DMA + 6 wide TT ops
            w4 = lut_pool.tile([NPART, NCHUNK, W, 4], bf16, tag="w4")
            nc.sync.dma_start(
                w4[:], lutp_d[:].rearrange("p c (w k) -> p c w k", k=4))

            # 7 full chunks + 4 small tail chunks (short post-gather tail);
            # one gather tile per chunk so dependency ranges stay per-chunk
            clims = [0, 32, 64, 96, 128, 160, 192, 224, 232, 240, 248, 256]
            # last chunk writing each 8-slot psum window (for stop flags)
            lastw = {}
            for _c in range(len(clims) - 1):
                for _k in range((clims[_c + 1] - clims[_c]) // 8):
                    lastw[((clims[_c] % 32) + _k * 8) // 8] = _c
            x0t = [x0_pool.tile([NPART, clims[c + 1] - clims[c], EPAD], bf16,
                                tag=f"x0_{c}", name=f"x0_{c}") for c in range(len(clims) - 1)]
            x1t = [x1_pool.tile([NPART, clims[c + 1] - clims[c], EPAD], bf16,
                                tag=f"x1_{c}", name=f"x1_{c}") for c in range(len(clims) - 1)]

            # coefficient transform (values are 4x the true a,b,c,d;
            # folded back by the 0.25 scale at the end)
            ca = coef_pool.tile([NPART, WT], bf16, tag="ca")
            cb = coef_pool.tile([NPART, WT], bf16, tag="cb")
            cc = coef_pool.tile([NPART, WT], bf16, tag="cc")
            cd = coef_pool.tile([NPART, WT], bf16, tag="cd")
            t1 = coef_pool.tile([NPART, WT], bf16, tag="t1")
            t2 = coef_pool.tile([NPART, WT], bf16, tag="t2")
            w4f = w4[:].rearrange("p c w k -> p (c w) k")
            nc.vector.tensor_tensor(t1[:], w4f[:, :, 0], w4f[:, :, 3], Alu.add)
            nc.vector.tensor_tensor(t2[:], w4f[:, :, 1], w4f[:, :, 2], Alu.add)
            nc.vector.tensor_tensor(ca[:], t1[:], t2[:], Alu.add)
            nc.vector.tensor_tensor(cd[:], t1[:], t2[:], Alu.subtract)
            nc.vector.tensor_tensor(t1[:], w4f[:, :, 3], w4f[:, :, 0], Alu.subtract)
            nc.vector.tensor_tensor(t2[:], w4f[:, :, 1], w4f[:, :, 2], Alu.subtract)
            nc.vector.tensor_tensor(cb[:], t1[:], t2[:], Alu.add)
            nc.vector.tensor_tensor(cc[:], t1[:], t2[:], Alu.subtract)

            # constant-term reduce (one op over all chunks)
            areda = red_pool.tile([NPART, 1], bf16, tag="areda")
            with nc.allow_low_precision(reason="a-term bf16 for PE rhs"):
                nc.vector.tensor_reduce(
                    areda[:], ca[:],
                    Axis.X, Alu.add)

            # psum accumulator [64o, 32w, 64b]: 4 banks, one 8-w window
            # each; matmuls accumulate chunks, final reduce collapses w.
            ps = psum_pool.tile([OC, 4, 8, B], f32, tag="ps")
            psa = psum_pool.tile([OC, 1], f32, tag="psa")
            redw = [None] * 4

            GS = GIDX // 16   # idx columns per sub-gather
            qn = 0
            for c in range(len(clims) - 1):
                lo, hi = clims[c], clims[c + 1]
                wc = hi - lo
                for g in range(lo // GW, hi // GW):
                    i0 = g * GS
                    gl = g * GW - lo
                    nc.gpsimd.dma_gather(
                        x0t[c][:, gl:gl + GW, :], input_t[:],
                        idx0_sb[:, i0:i0 + GS], GIDX, GIDX, EPAD,
                        queue_num=qn % NQUEUES)
                    nc.gpsimd.dma_gather(
                        x1t[c][:, gl:gl + GW, :], input_t[:],
                        idx1_sb[:, i0:i0 + GS], GIDX, GIDX, EPAD,
                        queue_num=(qn + 1) % NQUEUES)
                    qn += 2

                # y = x0*(b + d*x1) + c*x1 in bf16; segment-sum via PE.
                # ACT compacts the padded gather tiles to contiguous
                # [128, wc, 64] (x1 first: the first DVE op needs only x1);
                # contiguous operands keep the DVE adds in 2x mode. Small
                # tail chunks skip compaction (keeps ACT off the tail path).
                if wc > 8:
                    x0v = m_pool.tile([NPART, wc, B], bf16, tag=f"x0c{wc}")
                    x1v = m_pool.tile([NPART, wc, B], bf16, tag=f"x1c{wc}")
                    nc.vector.tensor_copy(x1v[:], x1t[c][:, :, 0:B])
                    nc.vector.tensor_copy(x0v[:], x0t[c][:, :, 0:B])
                    x0v, x1v = x0v[:], x1v[:]
                else:
                    x0v = x0t[c][:, :, 0:B]
                    x1v = x1t[c][:, :, 0:B]
                csl = (slice(None), slice(lo, hi))
                u = m_pool.tile([NPART, wc, B], bf16, tag=f"u{wc}")
                bcb = cb[csl].unsqueeze(2).broadcast_to([NPART, wc, B])
                bcc = cc[csl].unsqueeze(2).broadcast_to([NPART, wc, B])
                bcd = cd[csl].unsqueeze(2).broadcast_to([NPART, wc, B])
                nc.vector.tensor_tensor(u[:], x1v, bcd, Alu.mult)
                nc.vector.tensor_tensor(u[:], u[:], bcb, Alu.add)
                nc.vector.tensor_tensor(u[:], u[:], x0v, Alu.mult)
                yv = m_pool.tile([NPART, wc, B], bf16, tag=f"yv{wc}")
                nc.vector.tensor_tensor(yv[:], x1v, bcc, Alu.mult)
                nc.vector.tensor_tensor(yv[:], yv[:], u[:], Alu.add)
                # psum windows: global slot mod 32, per-8-slot banks
                for k in range(wc // 8):
                    wk = ((lo % 32) + k * 8) // 8
                    nc.tensor.matmul(
                        ps[:, wk], pm_sb[:], yv[:, k * 8:(k + 1) * 8],
                        start=(c == 0), stop=(c == lastw[wk]))
                    if c == lastw[wk]:
                        rw = red_pool.tile([OC, B], f32, tag=f"red{wk}",
                                           name=f"red{wk}")
                        redw[wk] = rw
                        nc.vector.tensor_reduce(
                            rw[:], ps[:, wk].transpose([0, 2, 1]),
                            Axis.X, Alu.add)

            # a-term: psa[o] = sum_p pm[p,o] * areda[p]
            nc.tensor.matmul(psa[:], pm_sb[:], areda[:], start=True, stop=True)
            psa_sb = red_pool.tile([OC, 1], f32, tag="psa_sb")
            nc.vector.tensor_copy(psa_sb[:], psa[:])

            # combine the per-window partial sums
            r01 = red_pool.tile([OC, B], f32, tag="r01")
            nc.vector.tensor_tensor(r01[:], redw[0][:], redw[1][:], Alu.add)
            r23 = red_pool.tile([OC, B], f32, tag="r23")
            nc.vector.tensor_tensor(r23[:], redw[2][:], redw[3][:], Alu.add)
            red = red_pool.tile([OC, B], f32, tag="red")
            nc.vector.tensor_tensor(red[:], r01[:], r23[:], Alu.add)

            # out = 0.25*(red + psa) + bias
            out_sb = small_pool.tile([OC, B], f32, tag="out")
            nc.vector.tensor_scalar(out_sb[:], red[:], psa_sb[:], None, Alu.add)
            nc.vector.tensor_scalar(out_sb[:], out_sb[:], 0.25, bias_sb[:],
                                    Alu.mult, Alu.add)
            nc.sync.dma_start(out_d[:], out_sb[:])

    nc.compile()
    return nc


def _host_prep(input, input_mask, lut_weights, bias):
    import ml_dtypes
    input_t = np.zeros((IN, EPAD), dtype=ml_dtypes.bfloat16)
    input_t[:, 0:B] = input.T.astype(ml_dtypes.bfloat16)
    m0 = input_mask[0::2]
    m1 = input_mask[1::2]

    p = np.arange(NPART)
    c = np.arange(NCHUNK)
    w = np.arange(W)
    # core-local table index for (partition, chunk, within-partition slot)
    tau = ((p[:, None, None] // 2) * SEG + (p[:, None, None] % 2) * (SEG // 2)
           + c[None, :, None] * W + w[None, None, :])          # [128, NCHUNK, W]
    tau_cwp = np.ascontiguousarray(tau.transpose(1, 2, 0))     # [NCHUNK, W, 128]

    pm = np.zeros((NPART, OC), dtype=ml_dtypes.bfloat16)
    pm[p, p // 2] = 1.0

    def wrap_idx(vals):  # [NCHUNK, W, 128] gather order -> dma_gather layout
        # wrap each GIDX-index sub-gather separately (16-partition wrap)
        wrapped = vals.reshape(NCHUNK * GSUB, GIDX // 16, 16).transpose(0, 2, 1)
        wrapped = np.tile(wrapped, (1, 8, 1))                  # [NCHUNK*GSUB, 128, GIDX//16]
        wrapped = wrapped.reshape(NCHUNK, GSUB, NPART, GIDX // 16)
        return np.ascontiguousarray(
            wrapped.transpose(2, 0, 1, 3).reshape(NPART, -1)).astype(np.int16)

    in_maps = []
    for core in range(NCORES):
        g = core * TC + tau_cwp                                # global tables
        lutp = lut_weights[core * TC + tau]                    # [128, NCHUNK, W, 4]
        lutp = np.ascontiguousarray(
            lutp.reshape(NPART, NCHUNK, W * 4)
        ).astype(ml_dtypes.bfloat16)
        in_maps.append({
            "input_t": input_t,
            "idx0": wrap_idx(m0[g]),
            "idx1": wrap_idx(m1[g]),
            "lutp": lutp,
            "bias_sh": np.ascontiguousarray(
                bias[core * OC:(core + 1) * OC].reshape(OC, 1)
            ).astype(np.float32, copy=False),
            "pm": pm,
        })
    return in_maps


def get_program():
    if "nc" not in _CACHE:
        _CACHE["nc"] = _build_program()
    return _CACHE["nc"]


def run(input, input_mask, lut_weights, bias, trace=False):
    from concourse.bass_utils import run_bass_kernel_spmd

    nc = get_program()
    in_maps = _host_prep(np.asarray(input), np.asarray(input_mask),
                         np.asarray(lut_weights), np.asarray(bias))
    res = run_bass_kernel_spmd(nc, in_maps, list(range(NCORES)), trace=trace)
    out = np.concatenate([r["out_c"].T for r in res.results], axis=1)
    return out.astype(np.float32, copy=False), res


def kernel(input, input_mask, lut_weights, bias):
    out, _ = run(input, input_mask, lut_weights, bias)
    return out


# revision 21
# speedup vs baseline: 1.0027x; 1.0027x over previous
"""Trainium2 Bass kernel for the LUT-linear (embedding_lookup) problem.

Math: per_table[b,t] = sum_c lut[t,c] * prod_j (1 + s_{c,j} x_j)/2 with
x_0 = input[b, mask[2t]], x_1 = input[b, mask[2t+1]], K=2 (KK=4 corners).
Expanding the corner products (codes s in {-1,+1}):
    per_table = a_t + b_t x0 + c_t x1 + d_t x0 x1
    4a = w0+w1+w2+w3, 4b = -w0+w1-w2+w3, 4c = -w0-w1+w2+w3, 4d = w0-w1-w2+w3
out[b,o] = bias[o] + sum_{t in seg_o} per_table   (segments are 512 contiguous
tables per out-feature).

Device strategy (8 NeuronCores, table-sharded; input replicated):
  - per core: 32768 tables = 64 out-features. Gather x0/x1 columns with
    SWDGE dma_gather from a bf16 padded input^T [512, 128] (256B rows, the
    top 64 lanes zero); each descriptor moves all 64 batch values of one
    input feature into per-chunk SBUF tiles [128 part, wc, 128]
    (partition = table mod 128, free = batch). Gathers are 1024 indices
    each (ucode ring limit) spread round-robin over 4 SWDGE queues.
  - Work is split into 7 full chunks (32 slots) + 4 small tail chunks
    (8 slots) so the post-last-gather compute tail stays short. One
    gather tile PER CHUNK keeps dependency ranges chunk-local.
  - ACT (ScalarE) compacts each full chunk's padded [.,.,128] gather
    tiles to contiguous [128, wc, 64] (x1 first - the first DVE op only
    needs x1). DVE computes y = x0*(b + d*x1) + c*x1 in bf16; with
    contiguous operands the tensor_tensor ADDs run 2x (bf16 MULTs are
    1x - no 2x uop exists for TT mult on cayman). Small tail chunks
    skip compaction (ACT off the tail critical path).
  - All segment reductions run on the PE: per chunk, matmul
    psum[64o, 8w-window, 64b] += pm^T @ y accumulates across chunks
    (window = global slot mod 32); one final strided tensor_reduce over
    w collapses psum to [64, 64]. The constant term a reduces on DVE
    once (one op over all chunks) and joins via a tiny [128,1] matmul.
  - Host does only data-independent layout transforms (transpose, cast,
    permute, shard) and the final unshard.
  - Measured: ~186.1 us HW exec (baseline of this architecture was
    186.99 us), rel err 4.2e-3. Breakdown: head ~16.7 us (runtime
    preamble ~3.5 + mlp library LOAD_LIB + lazy Q7 IRAM load ~6 + first
    gather latency), gather stream ~149 us, tail ~17 us, postamble.
  - The wall is the SWDGE gather stream: 128 dma_gather calls at
    ~2.33 ns/index = ~149 us. This is the Q7 CounterMachine descriptor
    push rate (TX and RX cores each push num_idxs 64B descriptors in
    parallel; cross-queue overlap is impossible because each extended
    instruction occupies all 8 Q7 cores and queue_num only selects the
    working pair). single_packet=True already coalesces the per-engine
    descriptor stream into one packet, so the SDMA small-descriptor HBM
    penalty is amortized; SDMA drain is NOT the wall. Dead ends
    explored: one-hot+PE gather (PE matmul instruction overhead
    ~427 ns/<=1024-col matmul makes it >= the SWDGE rate per table; DVE
    is_equal one-hot build alone costs 2.2 ns/table), ap_gather /
    indirect_copy (per-element Q7 compute gathers, ~100x too slow),
    custom 4-pair ucode (no Xtensa toolchain in container).
    NOTES: (1) x0/x1 pools live on SBUF side="right" - when they sit
    near the pinned DynamicDMAScratch carveout (addr 0), SDMA
    descriptor-ring reads conflict with gather-data writes (~218-226us).
    (2) tensor_reduce is always 1x; PSUM matmul output <= 1 bank
    (512 f32). (3) DVE clock 0.96 GHz; TT cost ~ (N/accel + 151)/0.96 ns.
"""

import numpy as np

NCORES = 8
B = 64
IN = 512
OUT = 512
T = IN * OUT
TC = T // NCORES          # tables per core = 32768
SEG = 512                 # tables per out-feature
OC = OUT // NCORES        # out-features per core = 64
NPART = 128
WT = TC // NPART          # tables per partition total = 256

# tuning knobs
NCHUNK = 8                # compute chunks per core
W = WT // NCHUNK          # tables per partition per chunk
TCHUNK = NPART * W        # tables per chunk
GIDX = 1024               # indices per dma_gather (ucode limit)
GSUB = TCHUNK // GIDX     # sub-gathers per compute chunk
GW = GIDX // NPART        # tables per partition per sub-gather
NQUEUES = 4
EPAD = 128                # padded batch row (bf16 elems) = 256B descriptor

_CACHE = {}


def _build_program():
    import concourse.bacc as bacc
    import concourse.mybir as mybir
    from concourse import library_config
    from concourse.tile import TileContext

    f32 = mybir.dt.float32
    bf16 = mybir.dt.bfloat16
    i16 = mybir.dt.int16
    Alu = mybir.AluOpType
    Axis = mybir.AxisListType

    S = TCHUNK // 16      # idx columns per chunk (16-partition wrap)

    nc = bacc.Bacc("TRN2", target_bir_lowering=False, debug=False,
                   num_devices=NCORES, num_swdge_queues=NQUEUES,
                   dynamic_dma_scratch_size=32768)

    input_t = nc.dram_tensor("input_t", [IN, EPAD], bf16, kind="ExternalInput")
    idx0_d = nc.dram_tensor("idx0", [NPART, NCHUNK * S], i16, kind="ExternalInput")
    idx1_d = nc.dram_tensor("idx1", [NPART, NCHUNK * S], i16, kind="ExternalInput")
    lutp_d = nc.dram_tensor("lutp", [NPART, NCHUNK, W * 4], bf16, kind="ExternalInput")
    bias_d = nc.dram_tensor("bias_sh", [OC, 1], f32, kind="ExternalInput")
    pm_d = nc.dram_tensor("pm", [NPART, OC], bf16, kind="ExternalInput")
    out_d = nc.dram_tensor("out_c", [OC, B], f32, kind="ExternalOutput")

    with TileContext(nc) as tc:
        nc.gpsimd.load_library(library_config.mlp)
        with (
            tc.tile_pool(name="idx", bufs=1) as idx_pool,
            tc.tile_pool(name="small", bufs=1) as small_pool,
            tc.tile_pool(name="lut", bufs=1) as lut_pool,
            tc.tile_pool(name="coef", bufs=1) as coef_pool,
            tc.tile_pool(name="x0", bufs=1, side="right") as x0_pool,
            tc.tile_pool(name="x1", bufs=1, side="right") as x1_pool,
            tc.tile_pool(name="m", bufs=2) as m_pool,
            tc.tile_pool(name="red", bufs=2) as red_pool,
            tc.tile_pool(name="psum", bufs=1, space="PSUM") as psum_pool,
        ):
            idx0_sb = idx_pool.tile([NPART, NCHUNK * S], i16, tag="idx0")
            idx1_sb = idx_pool.tile([NPART, NCHUNK * S], i16, tag="idx1")
            # per-chunk slice loads so the first gather only waits ~64KB
            for c in range(NCHUNK):
                sl = slice(c * S, (c + 1) * S)
                nc.sync.dma_start(idx0_sb[:, sl], idx0_d[:, sl])
                nc.sync.dma_start(idx1_sb[:, sl], idx1_d[:, sl])

            pm_sb = small_pool.tile([NPART, OC], bf16, tag="pm")
            nc.sync.dma_start(pm_sb[:], pm_d[:])
            bias_sb = small_pool.tile([OC, 1], f32, tag="bias")
            nc.sync.dma_start(bias_sb[:], bias_d[:])

            # full coefficient table, one DMA + 6 wide TT ops
            w4 = lut_pool.tile([NPART, NCHUNK, W, 4], bf16, tag="w4")
            nc.sync.dma_start(
                w4[:], lutp_d[:].rearrange("p c (w k) -> p c w k", k=4))

            # 7 full chunks + 4 small tail chunks (short post-gather tail);
            # one gather tile per chunk so dependency ranges stay per-chunk
            clims = [0, 32, 64, 96, 128, 160, 192, 224, 232, 240, 248, 256]
            # last chunk writing each 8-slot psum window (for stop flags)
            lastw = {}
            for _c in range(len(clims) - 1):
                for _k in range((clims[_c + 1] - clims[_c]) // 8):
                    lastw[((clims[_c] % 32) + _k * 8) // 8] = _c
            x0t = [x0_pool.tile([NPART, clims[c + 1] - clims[c], EPAD], bf16,
                                tag=f"x0_{c}", name=f"x0_{c}") for c in range(len(clims) - 1)]
            x1t = [x1_pool.tile([NPART, clims[c + 1] - clims[c], EPAD], bf16,
                                tag=f"x1_{c}", name=f"x1_{c}") for c in range(len(clims) - 1)]

            # coefficient transform (values are 4x the true a,b,c,d;
            # folded back by the 0.25 scale at the end)
            ca = coef_pool.tile([NPART, WT], bf16, tag="ca")
            cb = coef_pool.tile([NPART, WT], bf16, tag="cb")
            cc = coef_pool.tile([NPART, WT], bf16, tag="cc")
            cd = coef_pool.tile([NPART, WT], bf16, tag="cd")
            t1 = coef_pool.tile([NPART, WT], bf16, tag="t1")
            t2 = coef_pool.tile([NPART, WT], bf16, tag="t2")
            w4f = w4[:].rearrange("p c w k -> p (c w) k")
            nc.vector.tensor_tensor(t1[:], w4f[:, :, 0], w4f[:, :, 3], Alu.add)
            nc.vector.tensor_tensor(t2[:], w4f[:, :, 1], w4f[:, :, 2], Alu.add)
            nc.vector.tensor_tensor(ca[:], t1[:], t2[:], Alu.add)
            nc.vector.tensor_tensor(cd[:], t1[:], t2[:], Alu.subtract)
            nc.vector.tensor_tensor(t1[:], w4f[:, :, 3], w4f[:, :, 0], Alu.subtract)
            nc.vector.tensor_tensor(t2[:], w4f[:, :, 1], w4f[:, :, 2], Alu.subtract)
            nc.vector.tensor_tensor(cb[:], t1[:], t2[:], Alu.add)
            nc.vector.tensor_tensor(cc[:], t1[:], t2[:], Alu.subtract)

            # constant-term reduce (one op over all chunks)
            areda = red_pool.tile([NPART, 1], bf16, tag="areda")
            with nc.allow_low_precision(reason="a-term bf16 for PE rhs"):
                nc.vector.tensor_reduce(
                    areda[:], ca[:],
                    Axis.X, Alu.add)

            # psum accumulator [64o, 32w, 64b]: 4 banks, one 8-w window
            # each; matmuls accumulate chunks, final reduce collapses w.
            ps = psum_pool.tile([OC, 4, 8, B], f32, tag="ps")
            psa = psum_pool.tile([OC, 1], f32, tag="psa")
            redw = [None] * 4

            GS = GIDX // 16   # idx columns per sub-gather
            qn = 0
            for c in range(len(clims) - 1):
                lo, hi = clims[c], clims[c + 1]
                wc = hi - lo
                for g in range(lo // GW, hi // GW):
                    i0 = g * GS
                    gl = g * GW - lo
                    nc.gpsimd.dma_gather(
                        x0t[c][:, gl:gl + GW, :], input_t[:],
                        idx0_sb[:, i0:i0 + GS], GIDX, GIDX, EPAD,
                        queue_num=qn % NQUEUES)
                    nc.gpsimd.dma_gather(
                        x1t[c][:, gl:gl + GW, :], input_t[:],
                        idx1_sb[:, i0:i0 + GS], GIDX, GIDX, EPAD,
                        queue_num=(qn + 1) % NQUEUES)
                    qn += 2

                # y = x0*(b + d*x1) + c*x1 in bf16; segment-sum via PE.
                # ACT compacts the padded gather tiles to contiguous
                # [128, wc, 64] (x1 first: the first DVE op needs only x1);
                # contiguous operands keep the DVE adds in 2x mode. Small
                # tail chunks skip compaction (keeps ACT off the tail path).
                if wc > 8:
                    x0v = m_pool.tile([NPART, wc, B], bf16, tag=f"x0c{wc}")
                    x1v = m_pool.tile([NPART, wc, B], bf16, tag=f"x1c{wc}")
                    nc.scalar.copy(x1v[:], x1t[c][:, :, 0:B])
                    nc.scalar.copy(x0v[:], x0t[c][:, :, 0:B])
                    x0v, x1v = x0v[:], x1v[:]
                else:
                    x0v = x0t[c][:, :, 0:B]
                    x1v = x1t[c][:, :, 0:B]
                csl = (slice(None), slice(lo, hi))
                u = m_pool.tile([NPART, wc, B], bf16, tag=f"u{wc}")
                bcb = cb[csl].unsqueeze(2).broadcast_to([NPART, wc, B])
                bcc = cc[csl].unsqueeze(2).broadcast_to([NPART, wc, B])
                bcd = cd[csl].unsqueeze(2).broadcast_to([NPART, wc, B])
                nc.vector.tensor_tensor(u[:], x1v, bcd, Alu.mult)
                nc.vector.tensor_tensor(u[:], u[:], bcb, Alu.add)
                nc.vector.tensor_tensor(u[:], u[:], x0v, Alu.mult)
                yv = m_pool.tile([NPART, wc, B], bf16, tag=f"yv{wc}")
                nc.vector.tensor_tensor(yv[:], x1v, bcc, Alu.mult)
                nc.vector.tensor_tensor(yv[:], yv[:], u[:], Alu.add)
                # psum windows: global slot mod 32, per-8-slot banks
                for k in range(wc // 8):
                    wk = ((lo % 32) + k * 8) // 8
                    nc.tensor.matmul(
                        ps[:, wk], pm_sb[:], yv[:, k * 8:(k + 1) * 8],
                        start=(c == 0), stop=(c == lastw[wk]))
                    if c == lastw[wk]:
                        rw = red_pool.tile([OC, B], f32, tag=f"red{wk}",
                                           name=f"red{wk}")
                        redw[wk] = rw
                        nc.vector.tensor_reduce(
                            rw[:], ps[:, wk].transpose([0, 2, 1]),
                            Axis.X, Alu.add)

            # a-term: psa[o] = sum_p pm[p,o] * areda[p]
            nc.tensor.matmul(psa[:], pm_sb[:], areda[:], start=True, stop=True)
            psa_sb = red_pool.tile([OC, 1], f32, tag="psa_sb")
            nc.vector.tensor_copy(psa_sb[:], psa[:])

            # combine the per-window partial sums
            r01 = red_pool.tile([OC, B], f32, tag="r01")
            nc.vector.tensor_tensor(r01[:], redw[0][:], redw[1][:], Alu.add)
            r23 = red_pool.tile([OC, B], f32, tag="r23")
            nc.vector.tensor_tensor(r23[:], redw[2][:], redw[3][:], Alu.add)
            red = red_pool.tile([OC, B], f32, tag="red")
            nc.vector.tensor_tensor(red[:], r01[:], r23[:], Alu.add)

            # out = 0.25*(red + psa) + bias
            out_sb = small_pool.tile([OC, B], f32, tag="out")
            nc.vector.tensor_scalar(out_sb[:], red[:], psa_sb[:], None, Alu.add)
            nc.vector.tensor_scalar(out_sb[:], out_sb[:], 0.25, bias_sb[:],
                                    Alu.mult, Alu.add)
            nc.sync.dma_start(out_d[:], out_sb[:])

    nc.compile()
    return nc


def _host_prep(input, input_mask, lut_weights, bias):
    import ml_dtypes
    input_t = np.zeros((IN, EPAD), dtype=ml_dtypes.bfloat16)
    input_t[:, 0:B] = input.T.astype(ml_dtypes.bfloat16)
    m0 = input_mask[0::2]
    m1 = input_mask[1::2]

    p = np.arange(NPART)
    c = np.arange(NCHUNK)
    w = np.arange(W)
    # core-local table index for (partition, chunk, within-partition slot)
    tau = ((p[:, None, None] // 2) * SEG + (p[:, None, None] % 2) * (SEG // 2)
           + c[None, :, None] * W + w[None, None, :])          # [128, NCHUNK, W]
    tau_cwp = np.ascontiguousarray(tau.transpose(1, 2, 0))     # [NCHUNK, W, 128]

    pm = np.zeros((NPART, OC), dtype=ml_dtypes.bfloat16)
    pm[p, p // 2] = 1.0

    def wrap_idx(vals):  # [NCHUNK, W, 128] gather order -> dma_gather layout
        # wrap each GIDX-index sub-gather separately (16-partition wrap)
        wrapped = vals.reshape(NCHUNK * GSUB, GIDX // 16, 16).transpose(0, 2, 1)
        wrapped = np.tile(wrapped, (1, 8, 1))                  # [NCHUNK*GSUB, 128, GIDX//16]
        wrapped = wrapped.reshape(NCHUNK, GSUB, NPART, GIDX // 16)
        return np.ascontiguousarray(
            wrapped.transpose(2, 0, 1, 3).reshape(NPART, -1)).astype(np.int16)

    in_maps = []
    for core in range(NCORES):
        g = core * TC + tau_cwp                                # global tables
        lutp = lut_weights[core * TC + tau]                    # [128, NCHUNK, W, 4]
        lutp = np.ascontiguousarray(
            lutp.reshape(NPART, NCHUNK, W * 4)
        ).astype(ml_dtypes.bfloat16)
        in_maps.append({
            "input_t": input_t,
            "idx0": wrap_idx(m0[g]),
            "idx1": wrap_idx(m1[g]),
            "lutp": lutp,
            "bias_sh": np.ascontiguousarray(
                bias[core * OC:(core + 1) * OC].reshape(OC, 1)
            ).astype(np.float32, copy=False),
            "pm": pm,
        })
    return in_maps


def get_program():
    if "nc" not in _CACHE:
        _CACHE["nc"] = _build_program()
    return _CACHE["nc"]


def run(input, input_mask, lut_weights, bias, trace=False):
    from concourse.bass_utils import run_bass_kernel_spmd

    nc = get_program()
    in_maps = _host_prep(np.asarray(input), np.asarray(input_mask),
                         np.asarray(lut_weights), np.asarray(bias))
    res = run_bass_kernel_spmd(nc, in_maps, list(range(NCORES)), trace=trace)
    out = np.concatenate([r["out_c"].T for r in res.results], axis=1)
    return out.astype(np.float32, copy=False), res


def kernel(input, input_mask, lut_weights, bias):
    out, _ = run(input, input_mask, lut_weights, bias)
    return out


# revision 25
# speedup vs baseline: 1.0047x; 1.0020x over previous
"""Trainium2 Bass kernel for the LUT-linear (embedding_lookup) problem.

Math: per_table[b,t] = sum_c lut[t,c] * prod_j (1 + s_{c,j} x_j)/2 with
x_0 = input[b, mask[2t]], x_1 = input[b, mask[2t+1]], K=2 (KK=4 corners).
Expanding the corner products (codes s in {-1,+1}):
    per_table = a_t + b_t x0 + c_t x1 + d_t x0 x1
    4a = w0+w1+w2+w3, 4b = -w0+w1-w2+w3, 4c = -w0-w1+w2+w3, 4d = w0-w1-w2+w3
out[b,o] = bias[o] + sum_{t in seg_o} per_table   (segments are 512 contiguous
tables per out-feature).

Device strategy (8 NeuronCores, table-sharded; input replicated):
  - per core: 32768 tables = 64 out-features. Gather x0/x1 columns with
    SWDGE dma_gather from a bf16 padded input^T [512, 128] (256B rows, the
    top 64 lanes zero); each descriptor moves all 64 batch values of one
    input feature into per-chunk SBUF tiles [128 part, wc, 128]
    (partition = table mod 128, free = batch). Gathers are 1024 indices
    each (ucode ring limit) spread round-robin over 4 SWDGE queues.
  - Work is split into 7 full chunks (32 slots) + 4 small tail chunks
    (8 slots) so the post-last-gather compute tail stays short. One
    gather tile PER CHUNK keeps dependency ranges chunk-local.
  - ACT (ScalarE) compacts each full chunk's padded [.,.,128] gather
    tiles to contiguous [128, wc, 64] (x1 first - the first DVE op only
    needs x1). DVE computes y = x0*(b + d*x1) + c*x1 in bf16; with
    contiguous operands the tensor_tensor ADDs run 2x (bf16 MULTs are
    1x - no 2x uop exists for TT mult on cayman). Small tail chunks
    skip compaction (ACT off the tail critical path).
  - All segment reductions run on the PE: per chunk, matmul
    psum[64o, 8w-window, 64b] += pm^T @ y accumulates across chunks
    (window = global slot mod 32); one final strided tensor_reduce over
    w collapses psum to [64, 64]. The constant term a reduces on DVE
    once (one op over all chunks) and joins via a tiny [128,1] matmul.
  - Host does only data-independent layout transforms (transpose, cast,
    permute, shard) and the final unshard.
  - Measured: ~186.1 us HW exec (baseline of this architecture was
    186.99 us), rel err 4.2e-3. Breakdown: head ~16.7 us (runtime
    preamble ~3.5 + mlp library LOAD_LIB + lazy Q7 IRAM load ~6 + first
    gather latency), gather stream ~149 us, tail ~17 us, postamble.
  - The wall is the SWDGE gather stream: 128 dma_gather calls at
    ~2.33 ns/index = ~149 us. This is the Q7 CounterMachine descriptor
    push rate (TX and RX cores each push num_idxs 64B descriptors in
    parallel; cross-queue overlap is impossible because each extended
    instruction occupies all 8 Q7 cores and queue_num only selects the
    working pair). single_packet=True already coalesces the per-engine
    descriptor stream into one packet, so the SDMA small-descriptor HBM
    penalty is amortized; SDMA drain is NOT the wall. Dead ends
    explored: one-hot+PE gather (PE matmul instruction overhead
    ~427 ns/<=1024-col matmul makes it >= the SWDGE rate per table; DVE
    is_equal one-hot build alone costs 2.2 ns/table), ap_gather /
    indirect_copy (per-element Q7 compute gathers, ~100x too slow),
    custom 4-pair ucode (no Xtensa toolchain in container).
    NOTES: (1) x0/x1 pools live on SBUF side="right" - when they sit
    near the pinned DynamicDMAScratch carveout (addr 0), SDMA
    descriptor-ring reads conflict with gather-data writes (~218-226us).
    (2) tensor_reduce is always 1x; PSUM matmul output <= 1 bank
    (512 f32). (3) DVE clock 0.96 GHz; TT cost ~ (N/accel + 151)/0.96 ns.
"""

import numpy as np

NCORES = 8
B = 64
IN = 512
OUT = 512
T = IN * OUT
TC = T // NCORES          # tables per core = 32768
SEG = 512                 # tables per out-feature
OC = OUT // NCORES        # out-features per core = 64
NPART = 128
WT = TC // NPART          # tables per partition total = 256

# tuning knobs
NCHUNK = 8                # compute chunks per core
W = WT // NCHUNK          # tables per partition per chunk
TCHUNK = NPART * W        # tables per chunk
GIDX = 1024               # indices per dma_gather (ucode limit)
GSUB = TCHUNK // GIDX     # sub-gathers per compute chunk
GW = GIDX // NPART        # tables per partition per sub-gather
NQUEUES = 4
EPAD = 128                # padded batch row (bf16 elems) = 256B descriptor

_CACHE = {}


def _build_program():
    import concourse.bacc as bacc
    import concourse.mybir as mybir
    from concourse import library_config
    from concourse.tile import TileContext

    f32 = mybir.dt.float32
    bf16 = mybir.dt.bfloat16
    i16 = mybir.dt.int16
    Alu = mybir.AluOpType
    Axis = mybir.AxisListType

    S = TCHUNK // 16      # idx columns per chunk (16-partition wrap)

    nc = bacc.Bacc("TRN2", target_bir_lowering=False, debug=False,
                   num_devices=NCORES, num_swdge_queues=NQUEUES,
                   dynamic_dma_scratch_size=32768)

    input_t = nc.dram_tensor("input_t", [IN, EPAD], bf16, kind="ExternalInput")
    idx0_d = nc.dram_tensor("idx0", [NPART, NCHUNK * S], i16, kind="ExternalInput")
    idx1_d = nc.dram_tensor("idx1", [NPART, NCHUNK * S], i16, kind="ExternalInput")
    lutp_d = nc.dram_tensor("lutp", [NPART, NCHUNK, W * 4], bf16, kind="ExternalInput")
    bias_d = nc.dram_tensor("bias_sh", [OC, 1], f32, kind="ExternalInput")
    pm_d = nc.dram_tensor("pm", [NPART, OC], bf16, kind="ExternalInput")
    out_d = nc.dram_tensor("out_c", [OC, B], f32, kind="ExternalOutput")

    with TileContext(nc) as tc:
        nc.gpsimd.load_library(library_config.mlp)
        with (
            tc.tile_pool(name="idx", bufs=1) as idx_pool,
            tc.tile_pool(name="small", bufs=1) as small_pool,
            tc.tile_pool(name="lut", bufs=1) as lut_pool,
            tc.tile_pool(name="coef", bufs=1) as coef_pool,
            tc.tile_pool(name="x0", bufs=1, side="right") as x0_pool,
            tc.tile_pool(name="x1", bufs=1, side="right") as x1_pool,
            tc.tile_pool(name="m", bufs=2) as m_pool,
            tc.tile_pool(name="red", bufs=2) as red_pool,
            tc.tile_pool(name="psum", bufs=1, space="PSUM") as psum_pool,
        ):
            idx0_sb = idx_pool.tile([NPART, NCHUNK * S], i16, tag="idx0")
            idx1_sb = idx_pool.tile([NPART, NCHUNK * S], i16, tag="idx1")
            # per-chunk slice loads so the first gather only waits ~64KB
            for c in range(NCHUNK):
                sl = slice(c * S, (c + 1) * S)
                nc.sync.dma_start(idx0_sb[:, sl], idx0_d[:, sl])
                nc.sync.dma_start(idx1_sb[:, sl], idx1_d[:, sl])

            pm_sb = small_pool.tile([NPART, OC], bf16, tag="pm")
            nc.sync.dma_start(pm_sb[:], pm_d[:])
            bias_sb = small_pool.tile([OC, 1], f32, tag="bias")
            nc.sync.dma_start(bias_sb[:], bias_d[:])

            # full coefficient table, one DMA + 6 wide TT ops
            w4 = lut_pool.tile([NPART, NCHUNK, W, 4], bf16, tag="w4")
            nc.sync.dma_start(
                w4[:], lutp_d[:].rearrange("p c (w k) -> p c w k", k=4))

            # 7 full chunks + 4 small tail chunks (short post-gather tail);
            # one gather tile per chunk so dependency ranges stay per-chunk
            clims = [0, 32, 64, 96, 128, 160, 192, 224, 232, 240, 248, 256]
            # last chunk writing each 8-slot psum window (for stop flags)
            lastw = {}
            for _c in range(len(clims) - 1):
                for _k in range((clims[_c + 1] - clims[_c]) // 8):
                    lastw[((clims[_c] % 32) + _k * 8) // 8] = _c
            x0t = [x0_pool.tile([NPART, clims[c + 1] - clims[c], EPAD], bf16,
                                tag=f"x0_{c}", name=f"x0_{c}") for c in range(len(clims) - 1)]
            x1t = [x1_pool.tile([NPART, clims[c + 1] - clims[c], EPAD], bf16,
                                tag=f"x1_{c}", name=f"x1_{c}") for c in range(len(clims) - 1)]

            # coefficient transform (values are 4x the true a,b,c,d;
            # folded back by the 0.25 scale at the end)
            ca = coef_pool.tile([NPART, WT], bf16, tag="ca")
            cb = coef_pool.tile([NPART, WT], bf16, tag="cb")
            cc = coef_pool.tile([NPART, WT], bf16, tag="cc")
            cd = coef_pool.tile([NPART, WT], bf16, tag="cd")
            t1 = coef_pool.tile([NPART, WT], bf16, tag="t1")
            t2 = coef_pool.tile([NPART, WT], bf16, tag="t2")
            w4f = w4[:].rearrange("p c w k -> p (c w) k")
            nc.vector.tensor_tensor(t1[:], w4f[:, :, 0], w4f[:, :, 3], Alu.add)
            nc.vector.tensor_tensor(t2[:], w4f[:, :, 1], w4f[:, :, 2], Alu.add)
            nc.vector.tensor_tensor(ca[:], t1[:], t2[:], Alu.add)
            nc.vector.tensor_tensor(cd[:], t1[:], t2[:], Alu.subtract)
            nc.vector.tensor_tensor(t1[:], w4f[:, :, 3], w4f[:, :, 0], Alu.subtract)
            nc.vector.tensor_tensor(t2[:], w4f[:, :, 1], w4f[:, :, 2], Alu.subtract)
            nc.vector.tensor_tensor(cb[:], t1[:], t2[:], Alu.add)
            nc.vector.tensor_tensor(cc[:], t1[:], t2[:], Alu.subtract)

            # constant-term reduce (one op over all chunks)
            areda = red_pool.tile([NPART, 1], bf16, tag="areda")
            with nc.allow_low_precision(reason="a-term bf16 for PE rhs"):
                nc.vector.tensor_reduce(
                    areda[:], ca[:],
                    Axis.X, Alu.add)

            # psum accumulator [64o, 32w, 64b]: 4 banks, one 8-w window
            # each; matmuls accumulate chunks, final reduce collapses w.
            ps = psum_pool.tile([OC, 4, 8, B], f32, tag="ps")
            psa = psum_pool.tile([OC, 1], f32, tag="psa")
            redw = [None] * 4

            GS = GIDX // 16   # idx columns per sub-gather
            qn = 0
            for c in range(len(clims) - 1):
                lo, hi = clims[c], clims[c + 1]
                wc = hi - lo
                for g in range(lo // GW, hi // GW):
                    i0 = g * GS
                    gl = g * GW - lo
                    nc.gpsimd.dma_gather(
                        x0t[c][:, gl:gl + GW, :], input_t[:],
                        idx0_sb[:, i0:i0 + GS], GIDX, GIDX, EPAD,
                        queue_num=qn % NQUEUES)
                    nc.gpsimd.dma_gather(
                        x1t[c][:, gl:gl + GW, :], input_t[:],
                        idx1_sb[:, i0:i0 + GS], GIDX, GIDX, EPAD,
                        queue_num=(qn + 1) % NQUEUES)
                    qn += 2

                # y = x0*(b + d*x1) + c*x1 in bf16; segment-sum via PE.
                # ACT compacts the padded gather tiles to contiguous
                # [128, wc, 64] (x1 first: the first DVE op needs only x1);
                # contiguous operands keep the DVE adds in 2x mode. Small
                # tail chunks skip compaction (keeps ACT off the tail path).
                if wc > 8:
                    x0v = m_pool.tile([NPART, wc, B], bf16, tag=f"x0c{wc}")
                    x1v = m_pool.tile([NPART, wc, B], bf16, tag=f"x1c{wc}")
                    nc.scalar.copy(x1v[:], x1t[c][:, :, 0:B])
                    nc.scalar.copy(x0v[:], x0t[c][:, :, 0:B])
                    x0v, x1v = x0v[:], x1v[:]
                else:
                    x0v = x0t[c][:, :, 0:B]
                    x1v = x1t[c][:, :, 0:B]
                csl = (slice(None), slice(lo, hi))
                u = m_pool.tile([NPART, wc, B], bf16, tag=f"u{wc}")
                bcb = cb[csl].unsqueeze(2).broadcast_to([NPART, wc, B])
                bcc = cc[csl].unsqueeze(2).broadcast_to([NPART, wc, B])
                bcd = cd[csl].unsqueeze(2).broadcast_to([NPART, wc, B])
                nc.vector.tensor_tensor(u[:], x1v, bcd, Alu.mult)
                nc.vector.tensor_tensor(u[:], u[:], bcb, Alu.add)
                nc.vector.tensor_tensor(u[:], u[:], x0v, Alu.mult)
                yv = m_pool.tile([NPART, wc, B], bf16, tag=f"yv{wc}")
                nc.vector.tensor_tensor(yv[:], x1v, bcc, Alu.mult)
                nc.vector.tensor_tensor(yv[:], yv[:], u[:], Alu.add)
                # psum windows: global slot mod 32, per-8-slot banks
                for k in range(wc // 8):
                    wk = ((lo % 32) + k * 8) // 8
                    nc.tensor.matmul(
                        ps[:, wk], pm_sb[:], yv[:, k * 8:(k + 1) * 8],
                        start=(c == 0), stop=(c == lastw[wk]))
                    if c == lastw[wk]:
                        rw = red_pool.tile([OC, B], f32, tag=f"red{wk}",
                                           name=f"red{wk}")
                        redw[wk] = rw
                        nc.vector.tensor_reduce(
                            rw[:], ps[:, wk].transpose([0, 2, 1]),
                            Axis.X, Alu.add)

            # a-term: psa[o] = sum_p pm[p,o] * areda[p]
            nc.tensor.matmul(psa[:], pm_sb[:], areda[:], start=True, stop=True)
            psa_sb = red_pool.tile([OC, 1], f32, tag="psa_sb")
            nc.vector.tensor_copy(psa_sb[:], psa[:])

            # combine the per-window partial sums
            r01 = red_pool.tile([OC, B], f32, tag="r01")
            nc.vector.tensor_tensor(r01[:], redw[0][:], redw[1][:], Alu.add)
            r23 = red_pool.tile([OC, B], f32, tag="r23")
            nc.vector.tensor_tensor(r23[:], redw[2][:], redw[3][:], Alu.add)
            red = red_pool.tile([OC, B], f32, tag="red")
            nc.vector.tensor_tensor(red[:], r01[:], r23[:], Alu.add)

            # out = 0.25*(red + psa) + bias
            out_sb = small_pool.tile([OC, B], f32, tag="out")
            nc.vector.tensor_scalar(out_sb[:], red[:], psa_sb[:], None, Alu.add)
            nc.vector.tensor_scalar(out_sb[:], out_sb[:], 0.25, bias_sb[:],
                                    Alu.mult, Alu.add)
            nc.sync.dma_start(out_d[:], out_sb[:])

    nc.compile()
    return nc


def _host_prep(input, input_mask, lut_weights, bias):
    import ml_dtypes
    input_t = np.zeros((IN, EPAD), dtype=ml_dtypes.bfloat16)
    input_t[:, 0:B] = input.T.astype(ml_dtypes.bfloat16)
    m0 = input_mask[0::2]
    m1 = input_mask[1::2]

    p = np.arange(NPART)
    c = np.arange(NCHUNK)
    w = np.arange(W)
    # core-local table index for (partition, chunk, within-partition slot)
    tau = ((p[:, None, None] // 2) * SEG + (p[:, None, None] % 2) * (SEG // 2)
           + c[None, :, None] * W + w[None, None, :])          # [128, NCHUNK, W]
    tau_cwp = np.ascontiguousarray(tau.transpose(1, 2, 0))     # [NCHUNK, W, 128]

    pm = np.zeros((NPART, OC), dtype=ml_dtypes.bfloat16)
    pm[p, p // 2] = 1.0

    def wrap_idx(vals):  # [NCHUNK, W, 128] gather order -> dma_gather layout
        # wrap each GIDX-index sub-gather separately (16-partition wrap)
        wrapped = vals.reshape(NCHUNK * GSUB, GIDX // 16, 16).transpose(0, 2, 1)
        wrapped = np.tile(wrapped, (1, 8, 1))                  # [NCHUNK*GSUB, 128, GIDX//16]
        wrapped = wrapped.reshape(NCHUNK, GSUB, NPART, GIDX // 16)
        return np.ascontiguousarray(
            wrapped.transpose(2, 0, 1, 3).reshape(NPART, -1)).astype(np.int16)

    in_maps = []
    for core in range(NCORES):
        g = core * TC + tau_cwp                                # global tables
        lutp = lut_weights[core * TC + tau]                    # [128, NCHUNK, W, 4]
        lutp = np.ascontiguousarray(
            lutp.reshape(NPART, NCHUNK, W * 4)
        ).astype(ml_dtypes.bfloat16)
        in_maps.append({
            "input_t": input_t,
            "idx0": wrap_idx(m0[g]),
            "idx1": wrap_idx(m1[g]),
            "lutp": lutp,
            "bias_sh": np.ascontiguousarray(
                bias[core * OC:(core + 1) * OC].reshape(OC, 1)
            ).astype(np.float32, copy=False),
            "pm": pm,
        })
    return in_maps


def get_program():
    if "nc" not in _CACHE:
        _CACHE["nc"] = _build_program()
    return _CACHE["nc"]


def run(input, input_mask, lut_weights, bias, trace=False):
    from concourse.bass_utils import run_bass_kernel_spmd

    nc = get_program()
    in_maps = _host_prep(np.asarray(input), np.asarray(input_mask),
                         np.asarray(lut_weights), np.asarray(bias))
    res = run_bass_kernel_spmd(nc, in_maps, list(range(NCORES)), trace=trace)
    out = np.concatenate([r["out_c"].T for r in res.results], axis=1)
    return out.astype(np.float32, copy=False), res


def kernel(input, input_mask, lut_weights, bias):
    out, _ = run(input, input_mask, lut_weights, bias)
    return out
